# revision 9
# baseline (speedup 1.0000x reference)
"""Trainium2 Bass kernel for ComplexAttention (v3).

Math (per (b,t) pair):
    cur2 = [cur_r, cur_i]                       # [2D]
    Q    = cur2 @ qW + qb                       # [D]
    K_s  = H_s @ kW + kb ; V_s = H_s @ vW + vb  # H = [hist_r, hist_i]  [S, 2D]
    sc_s = (Q . K_s) * scale * conf
    w    = softmax(sc) ; ctx = sum_s w_s V_s
    out  = cur + 0.1 * ctx (complex)

Rewrites (exact):
    Q . K_s = (cur2 @ (qW kW^T)) . H_s + (Q . kb); the (Q . kb) term is
    constant over s -> softmax-invariant -> dropped. conf*scale folds into
    the cur2 columns on the host. qb folds in via a rank-1 conf x (qb kW^T)
    matmul term. ctx = (sum_s e_s H_s) @ vW / sum_s e_s + vb; the vb term
    is added on the host after gathering.

Structure:
    - H in fp8e3m4, host-prearranged. Partition p = 4*j + s4 (j = pair in
      sub-batch, s4 = slot quarter); slot s = s4*8 + sg, sg = 8 groups on
      the free axis. One 8KB-line DMA per 32-pair sub-batch.
    - scores for sg < KD are DVE fused scalar_tensor_tensor ops against a
      replicated Qk row block (one [128,128] sel matmul per sub-batch).
    - scores for the last K_PE groups run on the PE against a second,
      TRANSPOSED fp8 copy of those slots (H_t, e on partitions):
      P[q', c] = sum_e Qk[q', e] H_t[e, c] with columns c = 4q + s4, so
      transposing P puts exp-weights exactly in the hbar partition layout;
      the diagonal (q'==q) is selected by the same mask multiply that
      builds the block-diagonal hbar weights.
    - hbar: 16 accumulating [128,32]x[128,512] matmuls (f16 wd x fp8 H).
    - den: mask32 matmul over exp8 + per-PE-group ones matmuls; reciprocal
      applied as the ACT scale of the hbar PSUM->f16 copy.
"""

import os
import sys

import numpy as np

os.environ.setdefault("MYCRO_LOCAL_CACHE", "1")

try:
    import concourse.bass as bass
except ImportError:  # pragma: no cover
    sys.path.insert(0, "/opt/trn_rl_repo")
    import concourse.bass as bass

import ml_dtypes
import concourse.mybir as mybir
import concourse.tile as tile
from concourse import bacc
from concourse.bass_utils import run_bass_kernel_spmd

F32 = mybir.dt.float32
F16 = mybir.dt.float16
F8 = mybir.dt.float8e3
AX = mybir.AluOpType
AF = mybir.ActivationFunctionType
FP8_DT = ml_dtypes.float8_e3m4

B, T, S, D = 4, 1024, 32, 512
D2 = 2 * D  # 1024, concat(real, imag) feature dim
E = 2 * D   # 1024, history feature dim
N_CORES = 8
PAIRS = B * T
SCALE = float(D) ** -0.5

K_PE = int(os.environ.get("K_PE_GROUPS", "3"))  # score groups on PE (0..8)
KD = 8 - K_PE                                   # score groups on DVE


def build(ppc: int) -> bass.Bass:
    """Build the per-core SPMD program for `ppc` pairs per core."""
    assert ppc % 128 == 0
    nb = ppc // 128      # batches of 128 pairs
    nsb = 4 * nb         # sub-batches of 32 pairs

    nc = bacc.Bacc("TRN2", target_bir_lowering=False)

    hbm_h = nc.declare_dram_parameter("hbm_h", [nsb, 128, 8, E], F8,
                                      isOutput=False)
    if K_PE:
        hbm_ht = nc.declare_dram_parameter(
            "hbm_ht", [nsb, 128, 8, 128 * K_PE], F8, isOutput=False)
    cur_ri = nc.declare_dram_parameter("cur_ri", [128, ppc // 128, 2, D],
                                       F16, isOutput=False)
    cur2t = nc.declare_dram_parameter("cur2t", [128, 8, ppc], F16,
                                      isOutput=False)
    confr = nc.declare_dram_parameter("confr", [1, ppc], F16, isOutput=False)
    wqk = nc.declare_dram_parameter("wqk", [128, 8, E], F16, isOutput=False)
    bqk = nc.declare_dram_parameter("bqk", [1, E], F16, isOutput=False)
    vw = nc.declare_dram_parameter("vw", [128, 8, E], F16, isOutput=False)
    sel4 = nc.declare_dram_parameter("sel4", [128, 4, 128], F16, isOutput=False)
    mask8 = nc.declare_dram_parameter("mask8", [128, 8, 32], F16, isOutput=False)
    mask32 = nc.declare_dram_parameter("mask32", [128, 32], F16, isOutput=False)
    ones1 = nc.declare_dram_parameter("ones1", [128, 1], F16, isOutput=False)
    ident = nc.declare_dram_parameter("ident", [128, 128], F16, isOutput=False)
    out = nc.declare_dram_parameter("out", [128, ppc // 128, 2, D], F16,
                                    isOutput=True)

    from contextlib import ExitStack

    with tile.TileContext(nc) as tc, ExitStack() as es:
        ec_ = es.enter_context
        cpool = ec_(tc.tile_pool(name="const", bufs=1))
        wpool = ec_(tc.tile_pool(name="wqk", bufs=1))
        vpool = ec_(tc.tile_pool(name="vw", bufs=1))
        c2pool = ec_(tc.tile_pool(name="c2t", bufs=1))
        qkpool = ec_(tc.tile_pool(name="qk", bufs=2))
        qtpool = ec_(tc.tile_pool(name="qkT", bufs=2))
        hpool = ec_(tc.tile_pool(name="h", bufs=4))
        htpool2 = ec_(tc.tile_pool(name="ht8", bufs=4))
        qkspool = ec_(tc.tile_pool(name="qkrs", bufs=8))
        sinkpool = ec_(tc.tile_pool(name="sink", bufs=1))
        smpool = ec_(tc.tile_pool(name="sm", bufs=3))
        pspool = ec_(tc.tile_pool(name="Ps", bufs=2))
        etpool = ec_(tc.tile_pool(name="expT", bufs=2))
        wdpool = ec_(tc.tile_pool(name="wd", bufs=2))
        dnpool = ec_(tc.tile_pool(name="dn", bufs=2))
        hsbpool = ec_(tc.tile_pool(name="hsb", bufs=2))
        hbpool = ec_(tc.tile_pool(name="hbarb", bufs=2))
        htpool = ec_(tc.tile_pool(name="hbarT", bufs=8))
        curpool = ec_(tc.tile_pool(name="cur", bufs=2))
        outpool = ec_(tc.tile_pool(name="outp", bufs=2))
        ps_qkr = ec_(tc.tile_pool(name="ps_qkr", bufs=1, space="PSUM"))
        ps_hb = ec_(tc.tile_pool(name="ps_hb", bufs=1, space="PSUM"))
        ps_sh = ec_(tc.tile_pool(name="ps_sh", bufs=2, space="PSUM"))
        if K_PE:
            ps_sp = ec_(tc.tile_pool(name="ps_sp", bufs=1, space="PSUM"))
            ps_tp = ec_(tc.tile_pool(name="ps_tp", bufs=1, space="PSUM"))
        del es

        # ---- constants / weights, spread across DMA queues ----
        id_t = cpool.tile([128, 128], F16)
        nc.gpsimd.dma_start(out=id_t[:], in_=ident[:])
        sel_t = cpool.tile([128, 4, 128], F16)
        nc.gpsimd.dma_start(out=sel_t[:], in_=sel4[:])
        m8_t = cpool.tile([128, 8, 32], F16)
        nc.gpsimd.dma_start(out=m8_t[:], in_=mask8[:])
        m32_t = cpool.tile([128, 32], F16)
        nc.gpsimd.dma_start(out=m32_t[:], in_=mask32[:])
        ones_t = cpool.tile([128, 1], F16)
        nc.gpsimd.dma_start(out=ones_t[:], in_=ones1[:])
        bqk_t = cpool.tile([1, E], F16)
        nc.gpsimd.dma_start(out=bqk_t[:], in_=bqk[:])
        confr_t = cpool.tile([1, ppc], F16)
        nc.gpsimd.dma_start(out=confr_t[:], in_=confr[:])
        c2t_t = c2pool.tile([128, 8, ppc], F16)
        nc.sync.dma_start(out=c2t_t[:], in_=cur2t[:])
        wqk_t = wpool.tile([128, 8, E], F16)
        nc.sync.dma_start(out=wqk_t[:], in_=wqk[:])
        vw_t = vpool.tile([128, 8, E], F16)

        # ---- phase A (per batch): Qk = conf*scale*(cur2 @ Wqk + bqk) ----
        qks: dict[int, object] = {}
        qkTs: dict[int, object] = {}
        qkrs_s: dict[tuple, object] = {}

        def emit_A(b):
            qk_t = qkpool.tile([128, E], F16, tag="qk")
            for h in range(2):
                ps = ps_sh.tile([128, 512], F32, tag="mm512")
                for k in range(8):
                    nc.tensor.matmul(
                        ps[:],
                        lhsT=c2t_t[:, k, 128 * b : 128 * (b + 1)],
                        rhs=wqk_t[:, k, 512 * h : 512 * (h + 1)],
                        start=(k == 0),
                        stop=False,
                    )
                nc.tensor.matmul(
                    ps[:],
                    lhsT=confr_t[:, 128 * b : 128 * (b + 1)],
                    rhs=bqk_t[:, 512 * h : 512 * (h + 1)],
                    start=False,
                    stop=True,
                )
                nc.scalar.activation(
                    qk_t[:, 512 * h : 512 * (h + 1)], ps[:], AF.Copy
                )
            qks[b] = qk_t
            if KD:
                # replicate Qk rows for all 4 sub-batches up front:
                # qkr[4j+s4, e] = Qk[32*sb + j, e]
                for sb in range(4):
                    qkr = ps_qkr.tile([128, E], F32)
                    for h in range(2):
                        nc.tensor.matmul(
                            qkr[:, 512 * h : 512 * (h + 1)],
                            lhsT=sel_t[:, sb, :],
                            rhs=qk_t[:, 512 * h : 512 * (h + 1)],
                            start=True,
                            stop=True,
                        )
                    qkr_s = qkspool.tile([128, E], F16, tag="qkrs")
                    nc.scalar.activation(qkr_s[:], qkr[:], AF.Copy)
                    qkrs_s[(b, sb)] = qkr_s
            if K_PE:
                # QkT[e_p, ec, q] = Qk[q, 128*ec + e_p]
                tp = ps_sh.tile([128, 8, 128], F16, tag="mm512")
                for c in range(8):
                    nc.tensor.transpose(
                        tp[:, c, :], qk_t[:, 128 * c : 128 * (c + 1)], id_t[:]
                    )
                qkT_t = qtpool.tile([128, 8, 128], F16, tag="qkT")
                nc.scalar.activation(qkT_t[:], tp[:], AF.Copy)
                qkTs[b] = qkT_t

        # ---- phases B (scores/softmax/hbar) + C (ctx/out) ----
        batch_st: dict[int, dict] = {}

        h_tiles: dict[int, object] = {}
        ht_tiles: dict[int, object] = {}

        def emit_h_dma(sbi):
            b = sbi // 4
            if sbi % 4 == 0:
                cur_t = curpool.tile([128, 2, D], F16, tag="cur")
                nc.gpsimd.dma_start(out=cur_t[:], in_=cur_ri[:, b])
                hbar_b = hbpool.tile([128, E], F16)
                batch_st[b] = {"cur": cur_t, "hbar": hbar_b}
            h_t = hpool.tile([128, 8, E], F8, tag="h")
            nc.sync.dma_start(out=h_t[:], in_=hbm_h[sbi])
            h_tiles[sbi] = h_t
            if K_PE:
                ht_t = htpool2.tile([128, 8, 128 * K_PE], F8, tag="ht8")
                nc.sync.dma_start(out=ht_t[:], in_=hbm_ht[sbi])
                ht_tiles[sbi] = ht_t

        def emit_scores(b, sb):
            sbi = 4 * b + sb
            h_t = h_tiles.pop(sbi)

            st = {"b": b, "sb": sb, "h": h_t}

            if K_PE:
                ht_t = ht_tiles.pop(sbi)
                # P[q', c=(4q+s4)] = sum_e Qk[q', e] H[(q,s4), slot(sg), e]
                sP = ps_sp.tile([32, 128 * K_PE], F32)
                for c in range(8):
                    nc.tensor.matmul(
                        sP[:],
                        lhsT=qkTs[b][:, c, 32 * sb : 32 * (sb + 1)],
                        rhs=ht_t[:, c, :],
                        start=(c == 0),
                        stop=(c == 7),
                    )
                Ps = pspool.tile([32, 128 * K_PE], F16, tag="Ps")
                nc.scalar.activation(Ps[:], sP[:], AF.Copy)
                # transpose each 128-col block: T[(4q+s4), ki, q'] = P
                tp = ps_tp.tile([128, K_PE, 32], F16)
                for ki in range(K_PE):
                    nc.tensor.transpose(
                        tp[:, ki, :], Ps[:, 128 * ki : 128 * (ki + 1)],
                        id_t[0:32, 0:32],
                    )
                expT = etpool.tile([128, K_PE, 32], F16, tag="expT")
                nc.scalar.activation(expT[:], tp[:], AF.Exp)
                st["expT"] = expT

            if KD:
                qkr_s = qkrs_s.pop((b, sb))

                scores8 = smpool.tile([128, KD], F32, tag="scores")
                sink = sinkpool.tile([128, E], F16, tag="sink")
                for sg in range(KD):
                    nc.vector.scalar_tensor_tensor(
                        out=sink[:],
                        in0=h_t[:, sg, :],
                        scalar=1.0,
                        in1=qkr_s[:],
                        op0=AX.mult,
                        op1=AX.mult,
                        accum_out=scores8[:, sg : sg + 1],
                    )
                exp8 = smpool.tile([128, KD], F16, tag="exp")
                nc.scalar.activation(exp8[:], scores8[:], AF.Exp)
                st["exp8"] = exp8
            return st

        def emit_hbar(st):
            b, sb, h_t = st["b"], st["sb"], st["h"]
            hbar_b = batch_st[b]["hbar"]
            # wd[p, sg, q] = exp(score[p, sg]) iff q == p//4 (else 0)
            wd = wdpool.tile([128, 8, 32], F16, tag="wd")
            if KD:
                nc.gpsimd.tensor_tensor(
                    out=wd[:, 0:KD, :],
                    in0=m8_t[:, 0:KD, :],
                    in1=st["exp8"][:].unsqueeze(2).broadcast_to([128, KD, 32]),
                    op=AX.mult,
                )
            if K_PE:
                nc.gpsimd.tensor_tensor(
                    out=wd[:, KD:8, :],
                    in0=m8_t[:, KD:8, :],
                    in1=st["expT"][:],
                    op=AX.mult,
                )
            # hb32[q, e] = sum_{s4,sg} wd[(q,s4),sg] * H[(q,s4),sg,e]
            hb32 = ps_hb.tile([32, 2, 512], F32)
            for h in range(2):
                for sg in range(8):
                    nc.tensor.matmul(
                        hb32[:, h, :],
                        lhsT=wd[:, sg, :],
                        rhs=h_t[:, sg, 512 * h : 512 * (h + 1)],
                        start=(sg == 0),
                        stop=(sg == 7),
                    )
            # den[q] = total exp sum: DVE groups via mask32 x exp8,
            # PE groups via wd x ones.
            dn32 = ps_sh.tile([32, 8], F32, tag="mm512")
            if KD:
                nc.tensor.matmul(dn32[:, 0:KD], lhsT=m32_t[:],
                                 rhs=st["exp8"][:], start=True, stop=True)
            for ki in range(K_PE):
                nc.tensor.matmul(dn32[:, KD + ki : KD + ki + 1],
                                 lhsT=wd[:, KD + ki, :], rhs=ones_t[:],
                                 start=True, stop=True)
            d32 = dnpool.tile([32, 1], F32, tag="d32")
            nc.vector.tensor_reduce(out=d32[:], in_=dn32[:],
                                    axis=mybir.AxisListType.X, op=AX.add)
            inv32 = dnpool.tile([32, 1], F32, tag="inv32")
            nc.vector.reciprocal(inv32[:], d32[:])
            hsb32 = hsbpool.tile([32, E], F16)
            nc.scalar.activation(hsb32[:], hb32[:].rearrange("q a b -> q (a b)"),
                                 AF.Copy, scale=inv32[:])
            nc.gpsimd.dma_start(
                out=hbar_b[32 * sb : 32 * (sb + 1), :], in_=hsb32[:]
            )

        def emit_batch_end(b):
            hbar_b = batch_st[b]["hbar"]
            cur_t = batch_st[b]["cur"]
            hts = []
            for c in range(8):
                tp = ps_sh.tile([128, 128], F16, tag="mm512")
                nc.tensor.transpose(
                    tp[:], hbar_b[:, 128 * c : 128 * (c + 1)], id_t[:]
                )
                ht = htpool.tile([128, 128], F16, tag="hbarT")
                nc.scalar.activation(ht[:], tp[:], AF.Copy)
                hts.append(ht)

            out_t = outpool.tile([128, 2, D], F16)
            for h2 in range(2):
                cps = ps_sh.tile([128, 512], F32, tag="mm512")
                for c in range(8):
                    nc.tensor.matmul(
                        cps[:],
                        lhsT=hts[c][:],
                        rhs=vw_t[:, c, 512 * h2 : 512 * (h2 + 1)],
                        start=(c == 0),
                        stop=(c == 7),
                    )
                nc.vector.scalar_tensor_tensor(
                    out=out_t[:, h2, :],
                    in0=cps[:],
                    scalar=0.1,
                    in1=cur_t[:, h2, :],
                    op0=AX.mult,
                    op1=AX.add,
                )
            nc.gpsimd.dma_start(out=out[:, b], in_=out_t[:])

        LOOKAHEAD = 3
        emit_h_dma(0)
        emit_A(0)
        for i in range(1, min(LOOKAHEAD, nsb)):
            emit_h_dma(i)
        nc.sync.dma_start(out=vw_t[:], in_=vw[:])
        pend = None
        for b in range(nb):
            for sb in range(4):
                sbi = 4 * b + sb
                if sbi + LOOKAHEAD < nsb:
                    emit_h_dma(sbi + LOOKAHEAD)
                st = emit_scores(b, sb)
                if pend is not None:
                    emit_hbar(pend)
                    if pend["sb"] == 3:
                        emit_batch_end(pend["b"])
                if sb == 1 and b + 1 < nb:
                    emit_A(b + 1)
                pend = st
        emit_hbar(pend)
        emit_batch_end(pend["b"])

    nc.compile()
    return nc


_CACHE: dict[int, bass.Bass] = {}


def get_nc(ppc: int) -> bass.Bass:
    if ppc not in _CACHE:
        _CACHE[ppc] = build(ppc)
    return _CACHE[ppc]


def make_const_inputs():
    # sel4[k, sb, p] = 1 iff k == 32*sb + p//4
    sel_h = np.zeros((128, 4, 128), np.float16)
    for sb in range(4):
        for p in range(128):
            sel_h[32 * sb + p // 4, sb, p] = 1.0
    # mask8[p, sg, q] = 1 iff q == p//4 (same for all sg)
    mask8_h = np.zeros((128, 8, 32), np.float16)
    for p in range(128):
        mask8_h[p, :, p // 4] = 1.0
    mask32_h = np.ascontiguousarray(mask8_h[:, 0, :])
    id_h = np.eye(128, dtype=np.float16)
    return sel_h, mask8_h, mask32_h, id_h


def host_prep(hist_real, hist_imag, current_real, current_imag, confidence,
              qW, qb, kW, kb, vW, vb, ppc):
    """Host-side folding, fp8 conversion, per-core input maps."""
    f = lambda x: np.asarray(x, dtype=np.float32)
    current_real, current_imag = f(current_real), f(current_imag)
    confidence = f(confidence)
    qW, qb, kW, kb, vW, vb = f(qW), f(qb), f(kW), f(kb), f(vW), f(vb)

    n_cores = (B * T) // ppc
    nsb_tot = (B * T) // 32
    nsb = ppc // 32
    wqk_h = np.ascontiguousarray(
        (qW @ kW.T).astype(np.float16).reshape(8, 128, E).transpose(1, 0, 2))
    bqk_h = (qb @ kW.T).astype(np.float16).reshape(1, E)           # [1, E]
    vw_h = np.ascontiguousarray(
        vW.astype(np.float16).reshape(8, 128, E).transpose(1, 0, 2))
    sel_h, mask8_h, mask32_h, id_h = make_const_inputs()

    hr = np.asarray(hist_real, np.float32).reshape(B * T, S, D)
    hi = np.asarray(hist_imag, np.float32).reshape(B * T, S, D)
    cr = current_real.reshape(B * T, D)
    ci = current_imag.reshape(B * T, D)
    cf = confidence.reshape(B * T)

    # H_n fp8: [nsb, p=4j+s4, sg, e], slot s = s4*8 + sg
    hr5 = hr.reshape(nsb_tot, 32, 4, 8, D)
    hi5 = hi.reshape(nsb_tot, 32, 4, 8, D)
    h6 = np.concatenate([hr5, hi5], axis=-1)          # [nsb, j, s4, sg, E]
    hbm = np.ascontiguousarray(h6.reshape(nsb_tot, 128, 8, E)).astype(FP8_DT)

    if K_PE:
        # H_t fp8: [nsb, e_p, ec, c=(128*ki + 4j + s4)] for sg = KD + ki
        a = hbm.reshape(nsb_tot, 32, 4, 8, 8, 128)    # [sbi,j,s4,sg,ec,e_p]
        ht = a[:, :, :, KD:8].transpose(0, 5, 4, 3, 1, 2)
        hbm_ht = np.ascontiguousarray(
            ht.reshape(nsb_tot, 128, 8, 128 * K_PE))

    in_maps = []
    for c in range(n_cores):
        sl = slice(c * ppc, (c + 1) * ppc)
        cfs = cf[sl] * SCALE                          # [ppc]
        cur2t_h = np.ascontiguousarray(
            (np.concatenate([cr[sl], ci[sl]], axis=1) * cfs[:, None]).T
            .reshape(8, 128, ppc).transpose(1, 0, 2)
        ).astype(np.float16)  # [128, kc, ppc]
        cur_ri_h = np.ascontiguousarray(
            np.stack([cr[sl], ci[sl]], axis=1).astype(np.float16)
            .reshape(ppc // 128, 128, 2, D).transpose(1, 0, 2, 3))
        sbsl = slice(c * nsb, (c + 1) * nsb)
        m = {
            "hbm_h": np.ascontiguousarray(hbm[sbsl]),
            "cur_ri": cur_ri_h,
            "cur2t": cur2t_h,
            "confr": np.ascontiguousarray(
                (cfs.reshape(1, ppc)).astype(np.float16)),
            "wqk": wqk_h,
            "bqk": bqk_h,
            "vw": vw_h,
            "sel4": sel_h,
            "mask8": mask8_h,
            "mask32": mask32_h,
            "ones1": np.ones((128, 1), np.float16),
            "ident": id_h,
        }
        if K_PE:
            m["hbm_ht"] = np.ascontiguousarray(hbm_ht[sbsl])
        in_maps.append(m)
    return in_maps


def postprocess(out_full, vb):
    """out_full [n_cores*128, nb, 2, D] f16 (p-major per core) -> complex64."""
    vb = np.asarray(vb, dtype=np.float32)
    nbc = out_full.shape[1]
    o = (out_full.astype(np.float32)
         .reshape(-1, 128, nbc, 2, D).transpose(0, 2, 1, 3, 4)
         .reshape(PAIRS, 2, D))
    o_r = o[:, 0, :] + 0.1 * vb[:D]
    o_i = o[:, 1, :] + 0.1 * vb[D:]
    return (o_r + 1j * o_i).astype(np.complex64).reshape(B, T, D)


def kernel(hist_real, hist_imag, current_real, current_imag, confidence,
           qW, qb, kW, kb, vW, vb):
    ppc = PAIRS // N_CORES
    nc = get_nc(ppc)
    in_maps = host_prep(hist_real, hist_imag, current_real, current_imag,
                        confidence, qW, qb, kW, kb, vW, vb, ppc)
    res = run_bass_kernel_spmd(nc, in_maps, list(range(N_CORES))).results
    out = np.concatenate([res[c]["out"] for c in range(N_CORES)], axis=0)
    return postprocess(out, vb)


# revision 10
# speedup vs baseline: 1.1058x; 1.1058x over previous
"""Trainium2 Bass kernel for ComplexAttention (v3).

Math (per (b,t) pair):
    cur2 = [cur_r, cur_i]                       # [2D]
    Q    = cur2 @ qW + qb                       # [D]
    K_s  = H_s @ kW + kb ; V_s = H_s @ vW + vb  # H = [hist_r, hist_i]  [S, 2D]
    sc_s = (Q . K_s) * scale * conf
    w    = softmax(sc) ; ctx = sum_s w_s V_s
    out  = cur + 0.1 * ctx (complex)

Rewrites (exact):
    Q . K_s = (cur2 @ (qW kW^T)) . H_s + (Q . kb); the (Q . kb) term is
    constant over s -> softmax-invariant -> dropped. conf*scale folds into
    the cur2 columns on the host. qb folds in via a rank-1 conf x (qb kW^T)
    matmul term. ctx = (sum_s e_s H_s) @ vW / sum_s e_s + vb; the vb term
    is added on the host after gathering.

Structure:
    - H in fp8e3m4, host-prearranged. Partition p = 4*j + s4 (j = pair in
      sub-batch, s4 = slot quarter); slot s = s4*8 + sg, sg = 8 groups on
      the free axis. One 8KB-line DMA per 32-pair sub-batch.
    - scores for sg < KD are DVE fused scalar_tensor_tensor ops against a
      replicated Qk row block (one [128,128] sel matmul per sub-batch).
    - scores for the last K_PE groups run on the PE against a second,
      TRANSPOSED fp8 copy of those slots (H_t, e on partitions):
      P[q', c] = sum_e Qk[q', e] H_t[e, c] with columns c = 4q + s4, so
      transposing P puts exp-weights exactly in the hbar partition layout;
      the diagonal (q'==q) is selected by the same mask multiply that
      builds the block-diagonal hbar weights.
    - hbar: 16 accumulating [128,32]x[128,512] matmuls (f16 wd x fp8 H).
    - den: mask32 matmul over exp8 + per-PE-group ones matmuls; reciprocal
      applied as the ACT scale of the hbar PSUM->f16 copy.
"""

import os
import sys

import numpy as np

os.environ.setdefault("MYCRO_LOCAL_CACHE", "1")

try:
    import concourse.bass as bass
except ImportError:  # pragma: no cover
    sys.path.insert(0, "/opt/trn_rl_repo")
    import concourse.bass as bass

import ml_dtypes
import concourse.mybir as mybir
import concourse.tile as tile
from concourse import bacc
from concourse.bass_utils import run_bass_kernel_spmd

F32 = mybir.dt.float32
F16 = mybir.dt.float16
F8 = mybir.dt.float8e4
AX = mybir.AluOpType
AF = mybir.ActivationFunctionType
FP8_DT = ml_dtypes.float8_e4m3

B, T, S, D = 4, 1024, 32, 512
D2 = 2 * D  # 1024, concat(real, imag) feature dim
E = 2 * D   # 1024, history feature dim
N_CORES = 8
PAIRS = B * T
SCALE = float(D) ** -0.5

K_PE = int(os.environ.get("K_PE_GROUPS", "3"))
DR = mybir.MatmulPerfMode.DoubleRow  # score groups on PE (0..8)
KD = 8 - K_PE                                   # score groups on DVE


def build(ppc: int) -> bass.Bass:
    """Build the per-core SPMD program for `ppc` pairs per core."""
    assert ppc % 128 == 0
    nb = ppc // 128      # batches of 128 pairs
    nsb = 4 * nb         # sub-batches of 32 pairs

    nc = bacc.Bacc("TRN2", target_bir_lowering=False)

    hbm_h = nc.declare_dram_parameter("hbm_h", [nsb, 128, 8, E], F8,
                                      isOutput=False)
    if K_PE:
        hbm_ht = nc.declare_dram_parameter(
            "hbm_ht", [nsb, 128, 8, 128 * K_PE], F8, isOutput=False)
    cur_ri = nc.declare_dram_parameter("cur_ri", [128, ppc // 128, 2, D],
                                       F16, isOutput=False)
    cur2t = nc.declare_dram_parameter("cur2t", [128, 8, ppc], F16,
                                      isOutput=False)
    confr = nc.declare_dram_parameter("confr", [1, ppc], F16, isOutput=False)
    wqk = nc.declare_dram_parameter("wqk", [128, 8, E], F16, isOutput=False)
    bqk = nc.declare_dram_parameter("bqk", [1, E], F16, isOutput=False)
    vw = nc.declare_dram_parameter("vw", [128, 8, E], F16, isOutput=False)
    sel4 = nc.declare_dram_parameter("sel4", [128, 4, 128], F16, isOutput=False)
    mask8 = nc.declare_dram_parameter("mask8", [128, 8, 32], F16, isOutput=False)
    mask32 = nc.declare_dram_parameter("mask32", [128, 32], F16, isOutput=False)
    ones1 = nc.declare_dram_parameter("ones1", [128, 1], F8, isOutput=False)
    ident = nc.declare_dram_parameter("ident", [128, 128], F16, isOutput=False)
    out = nc.declare_dram_parameter("out", [128, ppc // 128, 2, D], F16,
                                    isOutput=True)

    from contextlib import ExitStack

    with tile.TileContext(nc) as tc, ExitStack() as es:
        ec_ = es.enter_context
        cpool = ec_(tc.tile_pool(name="const", bufs=1))
        wpool = ec_(tc.tile_pool(name="wqk", bufs=1))
        vpool = ec_(tc.tile_pool(name="vw", bufs=1))
        c2pool = ec_(tc.tile_pool(name="c2t", bufs=1))
        qkpool = ec_(tc.tile_pool(name="qk", bufs=2))
        qtpool = ec_(tc.tile_pool(name="qkT", bufs=2))
        hpool = ec_(tc.tile_pool(name="h", bufs=4))
        htpool2 = ec_(tc.tile_pool(name="ht8", bufs=4))
        qkspool = ec_(tc.tile_pool(name="qkrs", bufs=8))
        sinkpool = ec_(tc.tile_pool(name="sink", bufs=1))
        smpool = ec_(tc.tile_pool(name="sm", bufs=3))
        pspool = ec_(tc.tile_pool(name="Ps", bufs=2))
        etpool = ec_(tc.tile_pool(name="expT", bufs=2))
        wdpool = ec_(tc.tile_pool(name="wd", bufs=2))
        dnpool = ec_(tc.tile_pool(name="dn", bufs=2))
        hsbpool = ec_(tc.tile_pool(name="hsb", bufs=2))
        hbpool = ec_(tc.tile_pool(name="hbarb", bufs=2))
        htpool = ec_(tc.tile_pool(name="hbarT", bufs=8))
        curpool = ec_(tc.tile_pool(name="cur", bufs=2))
        outpool = ec_(tc.tile_pool(name="outp", bufs=2))
        ps_qkr = ec_(tc.tile_pool(name="ps_qkr", bufs=1, space="PSUM"))
        ps_hb = ec_(tc.tile_pool(name="ps_hb", bufs=1, space="PSUM"))
        ps_sh = ec_(tc.tile_pool(name="ps_sh", bufs=2, space="PSUM"))
        if K_PE:
            ps_sp = ec_(tc.tile_pool(name="ps_sp", bufs=1, space="PSUM"))
            ps_tp = ec_(tc.tile_pool(name="ps_tp", bufs=1, space="PSUM"))
        del es

        # ---- constants / weights, spread across DMA queues ----
        id_t = cpool.tile([128, 128], F16)
        nc.gpsimd.dma_start(out=id_t[:], in_=ident[:])
        sel_t = cpool.tile([128, 4, 128], F16)
        nc.gpsimd.dma_start(out=sel_t[:], in_=sel4[:])
        m8_t = cpool.tile([128, 8, 32], F16)
        nc.gpsimd.dma_start(out=m8_t[:], in_=mask8[:])
        m32_t = cpool.tile([128, 32], F16)
        nc.gpsimd.dma_start(out=m32_t[:], in_=mask32[:])
        ones_t = cpool.tile([128, 1], F8)
        nc.gpsimd.dma_start(out=ones_t[:], in_=ones1[:])
        bqk_t = cpool.tile([1, E], F16)
        nc.gpsimd.dma_start(out=bqk_t[:], in_=bqk[:])
        confr_t = cpool.tile([1, ppc], F16)
        nc.gpsimd.dma_start(out=confr_t[:], in_=confr[:])
        bias2 = cpool.tile([128, 1], F32)
        nc.gpsimd.memset(bias2[:], -2.0)
        c2t_t = c2pool.tile([128, 8, ppc], F16)
        nc.sync.dma_start(out=c2t_t[:], in_=cur2t[:])
        wqk_t = wpool.tile([128, 8, E], F16)
        nc.sync.dma_start(out=wqk_t[:], in_=wqk[:])
        vw_t = vpool.tile([128, 8, E], F16)

        # ---- phase A (per batch): Qk = conf*scale*(cur2 @ Wqk + bqk) ----
        qks: dict[int, object] = {}
        qkTs: dict[int, object] = {}
        qkrs_s: dict[tuple, object] = {}

        def emit_A(b):
            qk_t = qkpool.tile([128, E], F16, tag="qk")
            for h in range(2):
                ps = ps_sh.tile([128, 512], F32, tag="mm512")
                for k in range(8):
                    nc.tensor.matmul(
                        ps[:],
                        lhsT=c2t_t[:, k, 128 * b : 128 * (b + 1)],
                        rhs=wqk_t[:, k, 512 * h : 512 * (h + 1)],
                        start=(k == 0),
                        stop=False,
                    )
                nc.tensor.matmul(
                    ps[:],
                    lhsT=confr_t[:, 128 * b : 128 * (b + 1)],
                    rhs=bqk_t[:, 512 * h : 512 * (h + 1)],
                    start=False,
                    stop=True,
                )
                nc.scalar.activation(
                    qk_t[:, 512 * h : 512 * (h + 1)], ps[:], AF.Copy
                )
            qks[b] = qk_t
            if KD:
                # replicate Qk rows for all 4 sub-batches up front:
                # qkr[4j+s4, e] = Qk[32*sb + j, e]
                for sb in range(4):
                    qkr = ps_qkr.tile([128, E], F32)
                    for h in range(2):
                        nc.tensor.matmul(
                            qkr[:, 512 * h : 512 * (h + 1)],
                            lhsT=sel_t[:, sb, :],
                            rhs=qk_t[:, 512 * h : 512 * (h + 1)],
                            start=True,
                            stop=True,
                        )
                    qkr_s = qkspool.tile([128, E], F16, tag="qkrs")
                    nc.scalar.activation(qkr_s[:], qkr[:], AF.Copy)
                    qkrs_s[(b, sb)] = qkr_s
            if K_PE:
                # QkT[e_p, ec, q] = Qk[q, 128*ec + e_p]
                tp = ps_sh.tile([128, 8, 128], F16, tag="mm512")
                for c in range(8):
                    nc.tensor.transpose(
                        tp[:, c, :], qk_t[:, 128 * c : 128 * (c + 1)], id_t[:]
                    )
                qkT_t = qtpool.tile([128, 8, 128], F8, tag="qkT")
                nc.scalar.activation(qkT_t[:], tp[:], AF.Copy)
                qkTs[b] = qkT_t

        # ---- phases B (scores/softmax/hbar) + C (ctx/out) ----
        batch_st: dict[int, dict] = {}

        h_tiles: dict[int, object] = {}
        ht_tiles: dict[int, object] = {}

        def emit_h_dma(sbi):
            b = sbi // 4
            if sbi % 4 == 0:
                cur_t = curpool.tile([128, 2, D], F16, tag="cur")
                nc.gpsimd.dma_start(out=cur_t[:], in_=cur_ri[:, b])
                hbar_b = hbpool.tile([128, E], F16)
                batch_st[b] = {"cur": cur_t, "hbar": hbar_b}
            h_t = hpool.tile([128, 8, E], F8, tag="h")
            nc.sync.dma_start(out=h_t[:], in_=hbm_h[sbi])
            h_tiles[sbi] = h_t
            if K_PE:
                ht_t = htpool2.tile([128, 8, 128 * K_PE], F8, tag="ht8")
                nc.scalar.dma_start(out=ht_t[:], in_=hbm_ht[sbi])
                ht_tiles[sbi] = ht_t

        def emit_scores(b, sb):
            sbi = 4 * b + sb
            h_t = h_tiles.pop(sbi)

            st = {"b": b, "sb": sb, "h": h_t}

            if K_PE:
                ht_t = ht_tiles.pop(sbi)
                # P[q', c=(4q+s4)] = sum_e Qk[q', e] H[(q,s4), slot(sg), e]
                sP = ps_sp.tile([32, 128 * K_PE], F32)
                for cp in range(4):
                    nc.tensor.matmul(
                        sP[:],
                        lhsT=qkTs[b][:, 2 * cp : 2 * cp + 2,
                                     32 * sb : 32 * (sb + 1)],
                        rhs=ht_t[:, 2 * cp : 2 * cp + 2, :],
                        start=(cp == 0),
                        stop=(cp == 3),
                        perf_mode=DR,
                    )
                Ps = pspool.tile([32, 128 * K_PE], F16, tag="Ps")
                nc.scalar.activation(Ps[:], sP[:], AF.Copy)
                # transpose each 128-col block: T[(4q+s4), ki, q'] = P
                tp = ps_tp.tile([128, K_PE, 32], F16)
                for ki in range(K_PE):
                    nc.tensor.transpose(
                        tp[:, ki, :], Ps[:, 128 * ki : 128 * (ki + 1)],
                        id_t[0:32, 0:32],
                    )
                expT = etpool.tile([128, K_PE, 32], F16, tag="expT")
                nc.scalar.activation(expT[:], tp[:], AF.Exp, bias=bias2[:])
                st["expT"] = expT

            if KD:
                qkr_s = qkrs_s.pop((b, sb))

                scores8 = smpool.tile([128, KD], F32, tag="scores")
                sink = sinkpool.tile([128, E], F16, tag="sink")
                for sg in range(KD):
                    nc.vector.scalar_tensor_tensor(
                        out=sink[:],
                        in0=h_t[:, sg, :],
                        scalar=1.0,
                        in1=qkr_s[:],
                        op0=AX.mult,
                        op1=AX.mult,
                        accum_out=scores8[:, sg : sg + 1],
                    )
                exp8 = smpool.tile([128, KD], F16, tag="exp")
                nc.scalar.activation(exp8[:], scores8[:], AF.Exp,
                                     bias=bias2[:])
                st["exp8"] = exp8
            return st

        def emit_hbar(st):
            b, sb, h_t = st["b"], st["sb"], st["h"]
            hbar_b = batch_st[b]["hbar"]
            # wd[p, sg, q] = exp(score[p, sg]) iff q == p//4 (else 0)
            wd = wdpool.tile([128, 8, 32], F8, tag="wd")
            if KD:
                nc.gpsimd.tensor_tensor(
                    out=wd[:, 0:KD, :],
                    in0=m8_t[:, 0:KD, :],
                    in1=st["exp8"][:].unsqueeze(2).broadcast_to([128, KD, 32]),
                    op=AX.mult,
                )
            if K_PE:
                nc.gpsimd.tensor_tensor(
                    out=wd[:, KD:8, :],
                    in0=m8_t[:, KD:8, :],
                    in1=st["expT"][:],
                    op=AX.mult,
                )
            # hb32[q, e] = sum_{s4,sg} wd[(q,s4),sg] * H[(q,s4),sg,e]
            hb32 = ps_hb.tile([32, 2, 512], F32)
            for h in range(2):
                for sgp in range(4):
                    nc.tensor.matmul(
                        hb32[:, h, :],
                        lhsT=wd[:, 2 * sgp : 2 * sgp + 2, :],
                        rhs=h_t[:, 2 * sgp : 2 * sgp + 2,
                                512 * h : 512 * (h + 1)],
                        start=(sgp == 0),
                        stop=(sgp == 3),
                        perf_mode=DR,
                    )
            # den[q] = total exp sum: DVE groups via mask32 x exp8,
            # PE groups via wd x ones.
            dn32 = ps_sh.tile([32, 8], F32, tag="mm512")
            if KD:
                nc.tensor.matmul(dn32[:, 0:KD], lhsT=m32_t[:],
                                 rhs=st["exp8"][:], start=True, stop=True)
            for ki in range(K_PE):
                nc.tensor.matmul(dn32[:, KD + ki : KD + ki + 1],
                                 lhsT=wd[:, KD + ki, :], rhs=ones_t[:],
                                 start=True, stop=True)
            d32 = dnpool.tile([32, 1], F32, tag="d32")
            nc.vector.tensor_reduce(out=d32[:], in_=dn32[:],
                                    axis=mybir.AxisListType.X, op=AX.add)
            inv32 = dnpool.tile([32, 1], F32, tag="inv32")
            nc.vector.reciprocal(inv32[:], d32[:])
            hsb32 = hsbpool.tile([32, E], F16)
            nc.scalar.activation(hsb32[:], hb32[:].rearrange("q a b -> q (a b)"),
                                 AF.Copy, scale=inv32[:])
            nc.gpsimd.dma_start(
                out=hbar_b[32 * sb : 32 * (sb + 1), :], in_=hsb32[:]
            )

        def emit_batch_end(b):
            hbar_b = batch_st[b]["hbar"]
            cur_t = batch_st[b]["cur"]
            hts = []
            for c in range(8):
                tp = ps_sh.tile([128, 128], F16, tag="mm512")
                nc.tensor.transpose(
                    tp[:], hbar_b[:, 128 * c : 128 * (c + 1)], id_t[:]
                )
                ht = htpool.tile([128, 128], F16, tag="hbarT")
                nc.scalar.activation(ht[:], tp[:], AF.Copy)
                hts.append(ht)

            out_t = outpool.tile([128, 2, D], F16)
            for h2 in range(2):
                cps = ps_sh.tile([128, 512], F32, tag="mm512")
                for c in range(8):
                    nc.tensor.matmul(
                        cps[:],
                        lhsT=hts[c][:],
                        rhs=vw_t[:, c, 512 * h2 : 512 * (h2 + 1)],
                        start=(c == 0),
                        stop=(c == 7),
                    )
                nc.vector.scalar_tensor_tensor(
                    out=out_t[:, h2, :],
                    in0=cps[:],
                    scalar=0.1,
                    in1=cur_t[:, h2, :],
                    op0=AX.mult,
                    op1=AX.add,
                )
            nc.gpsimd.dma_start(out=out[:, b], in_=out_t[:])

        LOOKAHEAD = 3
        emit_h_dma(0)
        emit_A(0)
        for i in range(1, min(LOOKAHEAD, nsb)):
            emit_h_dma(i)
        nc.sync.dma_start(out=vw_t[:], in_=vw[:])
        pend = None
        for b in range(nb):
            for sb in range(4):
                sbi = 4 * b + sb
                if sbi + LOOKAHEAD < nsb:
                    emit_h_dma(sbi + LOOKAHEAD)
                st = emit_scores(b, sb)
                if pend is not None:
                    emit_hbar(pend)
                    if pend["sb"] == 3:
                        emit_batch_end(pend["b"])
                if sb == 1 and b + 1 < nb:
                    emit_A(b + 1)
                pend = st
        emit_hbar(pend)
        emit_batch_end(pend["b"])

    nc.compile()
    return nc


_CACHE: dict[int, bass.Bass] = {}


def get_nc(ppc: int) -> bass.Bass:
    if ppc not in _CACHE:
        _CACHE[ppc] = build(ppc)
    return _CACHE[ppc]


def make_const_inputs():
    # sel4[k, sb, p] = 1 iff k == 32*sb + p//4
    sel_h = np.zeros((128, 4, 128), np.float16)
    for sb in range(4):
        for p in range(128):
            sel_h[32 * sb + p // 4, sb, p] = 1.0
    # mask8[p, sg, q] = 1 iff q == p//4 (same for all sg)
    mask8_h = np.zeros((128, 8, 32), np.float16)
    for p in range(128):
        mask8_h[p, :, p // 4] = 1.0
    mask32_h = np.ascontiguousarray(mask8_h[:, 0, :])
    id_h = np.eye(128, dtype=np.float16)
    return sel_h, mask8_h, mask32_h, id_h


def host_prep(hist_real, hist_imag, current_real, current_imag, confidence,
              qW, qb, kW, kb, vW, vb, ppc):
    """Host-side folding, fp8 conversion, per-core input maps."""
    f = lambda x: np.asarray(x, dtype=np.float32)
    current_real, current_imag = f(current_real), f(current_imag)
    confidence = f(confidence)
    qW, qb, kW, kb, vW, vb = f(qW), f(qb), f(kW), f(kb), f(vW), f(vb)

    n_cores = (B * T) // ppc
    nsb_tot = (B * T) // 32
    nsb = ppc // 32
    wqk_h = np.ascontiguousarray(
        (qW @ kW.T).astype(np.float16).reshape(8, 128, E).transpose(1, 0, 2))
    bqk_h = (qb @ kW.T).astype(np.float16).reshape(1, E)           # [1, E]
    vw_h = np.ascontiguousarray(
        vW.astype(np.float16).reshape(8, 128, E).transpose(1, 0, 2))
    sel_h, mask8_h, mask32_h, id_h = make_const_inputs()

    hr = np.asarray(hist_real, np.float32).reshape(B * T, S, D)
    hi = np.asarray(hist_imag, np.float32).reshape(B * T, S, D)
    cr = current_real.reshape(B * T, D)
    ci = current_imag.reshape(B * T, D)
    cf = confidence.reshape(B * T)

    # H_n fp8: [nsb, p=4j+s4, sg, e], slot s = s4*8 + sg
    hr5 = hr.reshape(nsb_tot, 32, 4, 8, D)
    hi5 = hi.reshape(nsb_tot, 32, 4, 8, D)
    h6 = np.concatenate([hr5, hi5], axis=-1)          # [nsb, j, s4, sg, E]
    hbm = np.ascontiguousarray(h6.reshape(nsb_tot, 128, 8, E)).astype(FP8_DT)

    if K_PE:
        # H_t fp8: [nsb, e_p, ec, c=(128*ki + 4j + s4)] for sg = KD + ki
        a = hbm.reshape(nsb_tot, 32, 4, 8, 8, 128)    # [sbi,j,s4,sg,ec,e_p]
        ht = a[:, :, :, KD:8].transpose(0, 5, 4, 3, 1, 2)
        hbm_ht = np.ascontiguousarray(
            ht.reshape(nsb_tot, 128, 8, 128 * K_PE))

    in_maps = []
    for c in range(n_cores):
        sl = slice(c * ppc, (c + 1) * ppc)
        cfs = cf[sl] * SCALE                          # [ppc]
        cur2t_h = np.ascontiguousarray(
            (np.concatenate([cr[sl], ci[sl]], axis=1) * cfs[:, None]).T
            .reshape(8, 128, ppc).transpose(1, 0, 2)
        ).astype(np.float16)  # [128, kc, ppc]
        cur_ri_h = np.ascontiguousarray(
            np.stack([cr[sl], ci[sl]], axis=1).astype(np.float16)
            .reshape(ppc // 128, 128, 2, D).transpose(1, 0, 2, 3))
        sbsl = slice(c * nsb, (c + 1) * nsb)
        m = {
            "hbm_h": np.ascontiguousarray(hbm[sbsl]),
            "cur_ri": cur_ri_h,
            "cur2t": cur2t_h,
            "confr": np.ascontiguousarray(
                (cfs.reshape(1, ppc)).astype(np.float16)),
            "wqk": wqk_h,
            "bqk": bqk_h,
            "vw": vw_h,
            "sel4": sel_h,
            "mask8": mask8_h,
            "mask32": mask32_h,
            "ones1": np.ones((128, 1), FP8_DT),
            "ident": id_h,
        }
        if K_PE:
            m["hbm_ht"] = np.ascontiguousarray(hbm_ht[sbsl])
        in_maps.append(m)
    return in_maps


def postprocess(out_full, vb):
    """out_full [n_cores*128, nb, 2, D] f16 (p-major per core) -> complex64."""
    vb = np.asarray(vb, dtype=np.float32)
    nbc = out_full.shape[1]
    o = (out_full.astype(np.float32)
         .reshape(-1, 128, nbc, 2, D).transpose(0, 2, 1, 3, 4)
         .reshape(PAIRS, 2, D))
    o_r = o[:, 0, :] + 0.1 * vb[:D]
    o_i = o[:, 1, :] + 0.1 * vb[D:]
    return (o_r + 1j * o_i).astype(np.complex64).reshape(B, T, D)


def kernel(hist_real, hist_imag, current_real, current_imag, confidence,
           qW, qb, kW, kb, vW, vb):
    ppc = PAIRS // N_CORES
    nc = get_nc(ppc)
    in_maps = host_prep(hist_real, hist_imag, current_real, current_imag,
                        confidence, qW, qb, kW, kb, vW, vb, ppc)
    res = run_bass_kernel_spmd(nc, in_maps, list(range(N_CORES))).results
    out = np.concatenate([res[c]["out"] for c in range(N_CORES)], axis=0)
    return postprocess(out, vb)
